# revision 1
# baseline (speedup 1.0000x reference)
"""Hand-written Bass/Tile kernel for nn_Gtu2d on 8 TRN2 NeuronCores.

Data-parallel over batch (core i owns batch elem i). RPE MLP + Toeplitz
coefficient spectrum (Af) replicated per core (zero collectives),
staged through DRAM. All matmuls bf16 -> f32 PSUM. 2D rFFT/irFFT =
dense DFT matmuls; circulant embedding folded into the coefficient DFT
matrices as a phase shift.
"""

import numpy as np
import ml_dtypes

import concourse.bass as bass
import concourse.mybir as mybir
import concourse.tile as tile
from concourse.bass_utils import run_bass_kernel_spmd

BF16 = mybir.dt.bfloat16
F32 = mybir.dt.float32
AF = mybir.ActivationFunctionType

N = 56
T = N * N            # 3136
C0 = 512
D1 = 1024
P = 112
LW = 57
L2 = 114
GRID = P * P         # 12544
NCORES = 8
CQ = 256             # channel quarter
NQ = D1 // CQ
EPS = 1e-8
TN = 448
NT = T // TN

bf = ml_dtypes.bfloat16


def _np_consts():
    n = np.arange(N)
    k = np.arange(P)
    l = np.arange(LW)
    EWc = np.cos(-2 * np.pi * np.outer(n, l) / P).astype(np.float32)
    EWs = np.sin(-2 * np.pi * np.outer(n, l) / P).astype(np.float32)
    EW2 = np.concatenate([EWc, EWs], axis=1)
    EHc = np.cos(-2 * np.pi * np.outer(n, k) / P).astype(np.float32)
    EHs = np.sin(-2 * np.pi * np.outer(n, k) / P).astype(np.float32)
    pi_ = np.arange(P)
    GHc = np.cos(-2 * np.pi * np.outer(pi_ - 55, k) / P).astype(np.float32)
    GHs = np.sin(-2 * np.pi * np.outer(pi_ - 55, k) / P).astype(np.float32)
    GHc[111] = 0.0
    GHs[111] = 0.0
    GWc = np.cos(-2 * np.pi * np.outer(pi_ - 55, l) / P).astype(np.float32)
    GWs = np.sin(-2 * np.pi * np.outer(pi_ - 55, l) / P).astype(np.float32)
    GWc[111] = 0.0
    GWs[111] = 0.0
    GW2 = np.concatenate([GWc, GWs], axis=1)
    wl = np.full(LW, 2.0, np.float32)
    wl[0] = 1.0
    wl[-1] = 1.0
    IHc = np.cos(2 * np.pi * np.outer(k, n) / P).astype(np.float32)
    IHs = np.sin(2 * np.pi * np.outer(k, n) / P).astype(np.float32)
    IWc = (wl[:, None] * np.cos(2 * np.pi * np.outer(l, n) / P) / (P * P)).astype(np.float32)
    IWs = (wl[:, None] * np.sin(2 * np.pi * np.outer(l, n) / P) / (P * P)).astype(np.float32)
    IH_A = np.concatenate([IHc, IHs], axis=1)
    IH_B = np.concatenate([-IHs, IHc], axis=1)
    dp = np.arange(P, dtype=np.float32) - 55.0
    pp, qq = np.meshgrid(dp, dp, indexing='ij')
    coords = np.stack([pp.ravel(), qq.ravel(), np.ones(GRID, np.float32)])
    return dict(EW2=EW2, EHc=EHc, EHs=EHs, nEHs=-EHs, GW2=GW2,
                GHc=GHc, GHs=GHs, nGHs=-GHs,
                IH_A=IH_A, IH_B=IH_B, IWc=IWc, nIWs=-IWs, coordsT=coords)


_CONSTS = _np_consts()
_MATNAMES = ("EW2", "EHc", "EHs", "nEHs", "GW2", "GHc", "GHs", "nGHs",
             "IH_A", "IH_B", "IWc", "nIWs")


def build_nc():
    nc = bass.Bass()
    dp = {}

    def param(name, shape, dt=BF16):
        dp[name] = nc.declare_dram_parameter(name, list(shape), dt,
                                             isOutput=False)
        return dp[name]

    param("xT", (4, 128, T))
    param("wuT", (128, 4, D1))
    param("wvT", (128, 4, D1))
    param("woT", (128, 8, C0))
    param("bu", (128, 8), F32)
    param("bv", (128, 8), F32)
    param("bo", (128, 4), F32)
    param("coordsT", (3, GRID))
    param("rpe_inT", (3, C0))
    for i in range(3):
        param(f"rpe_hT{i}", (128, 4, C0))
    param("rpe_outT", (128, 4, D1))
    param("rpe_outB", (1, D1))
    param("rpe_hB", (128, 12), F32)
    for mname in _MATNAMES:
        param(mname, _CONSTS[mname].shape)
    param("ident", (128, 128))
    param("ones", (128, 128))
    outT = nc.declare_dram_parameter("outT", [4, 128, T], F32, isOutput=True)

    with tile.TileContext(nc, pool_alloc_mode="queue") as tc:
        with nc.allow_low_precision(reason="bf16 kernel, 2e-2 tolerance"), \
             tc.tile_pool(name="consts", bufs=1) as consts, \
             tc.tile_pool(name="persist", bufs=1) as persist, \
             tc.tile_pool(name="psum", bufs=4, space="PSUM") as psum, \
             tc.tile_pool(name="dram", bufs=1, space="DRAM") as dram:

            def cload(name, dt=BF16):
                t = consts.tile(list(dp[name].shape), dt, tag=name)
                nc.sync.dma_start(out=t, in_=dp[name][:])
                return t

            C = {m: cload(m) for m in _MATNAMES}
            ident = cload("ident")
            ones = cload("ones")
            wu_s = cload("wuT")
            wv_s = cload("wvT")
            wo_s = cload("woT")
            bu_s = cload("bu", F32)
            bv_s = cload("bv", F32)
            bo_s = cload("bo", F32)

            xnT = persist.tile([128, 4, T], BF16, tag="xnT")

            g_dram = dram.tile([GRID, D1], BF16, tag="g_dram")
            afr_dram = dram.tile([P, D1 * LW], BF16, tag="afr_dram")
            afi_dram = dram.tile([P, D1 * LW], BF16, tag="afi_dram")
            gat_dram = dram.tile([8, 128, T], BF16, tag="gat_dram")
            afr3 = afr_dram.rearrange("p (c l) -> p c l", l=LW)
            afi3 = afi_dram.rearrange("p (c l) -> p c l", l=LW)

            # ============ RPE trunk -> g_dram ============
            PCH = 1568
            QN = 4
            QW = PCH // QN
            rpe_cm = tc.tile_pool(name="rpew", bufs=1)
            rpw = rpe_cm.__enter__()

            def rload(name, dt=BF16):
                t = rpw.tile(list(dp[name].shape), dt, tag=name)
                nc.sync.dma_start(out=t, in_=dp[name][:])
                return t

            rin_s = rload("rpe_inT")
            rh_s = [rload(f"rpe_hT{i}") for i in range(3)]
            rout_s = rload("rpe_outT")
            routb_s = rload("rpe_outB")
            rhb_s = rload("rpe_hB", F32)
            for pc in range(GRID // PCH):
                pt_cm = tc.tile_pool(name="ptrunk", bufs=1)
                pt_pool = pt_cm.__enter__()
                cs = pt_pool.tile([3, PCH], BF16, tag="coords")
                nc.sync.dma_start(
                    out=cs, in_=dp["coordsT"][:, pc * PCH:(pc + 1) * PCH])
                tcur = pt_pool.tile([128, 4, PCH], BF16, tag="t_a")
                for mm in range(4):
                    for qn in range(QN):
                        ps = psum.tile([128, QW], F32, tag="ps")
                        nc.tensor.matmul(
                            ps, rin_s[:, mm * 128:(mm + 1) * 128],
                            cs[:, qn * QW:(qn + 1) * QW],
                            start=True, stop=True)
                        nc.scalar.activation(
                            out=tcur[:, mm, qn * QW:(qn + 1) * QW],
                            in_=ps, func=AF.Copy)

                def norm_relu(tin, tag):
                    s0 = pt_pool.tile([128, PCH], BF16, tag="nsq0")
                    tmp = pt_pool.tile([128, PCH], BF16, tag="nsq1")
                    nc.vector.tensor_mul(s0, tin[:, 0, :], tin[:, 0, :])
                    for cc in range(1, 4):
                        nc.vector.tensor_mul(tmp, tin[:, cc, :], tin[:, cc, :])
                        nc.vector.tensor_add(s0, s0, tmp)
                    wb = pt_pool.tile([128, PCH], BF16, tag="wb")
                    for qn in range(QN):
                        ps = psum.tile([128, QW], F32, tag="ps")
                        nc.tensor.matmul(ps, ones,
                                         s0[:, qn * QW:(qn + 1) * QW],
                                         start=True, stop=True)
                        rt = pt_pool.tile([128, QW], F32, tag="nrt")
                        nc.scalar.activation(out=rt, in_=ps, func=AF.Sqrt,
                                             scale=float(1.0 / C0))
                        nc.vector.reciprocal(
                            out=wb[:, qn * QW:(qn + 1) * QW], in_=rt)
                    acur = pt_pool.tile([128, 4, PCH], BF16, tag=tag)
                    for cc in range(4):
                        rl = pt_pool.tile([128, PCH], BF16, tag="nrl")
                        nc.scalar.activation(out=rl, in_=tin[:, cc, :],
                                             func=AF.Relu)
                        nc.vector.tensor_mul(acur[:, cc, :], rl, wb)
                    return acur, wb

                for li in range(3):
                    acur, _ = norm_relu(tcur, "t_b")
                    tnxt = pt_pool.tile([128, 4, PCH], BF16, tag="t_c")
                    for mm in range(4):
                        for qn in range(QN):
                            ps = psum.tile([128, QW], F32, tag="ps")
                            for kk in range(4):
                                nc.tensor.matmul(
                                    ps,
                                    rh_s[li][:, kk, mm * 128:(mm + 1) * 128],
                                    acur[:, kk, qn * QW:(qn + 1) * QW],
                                    start=(kk == 0), stop=(kk == 3))
                            nc.vector.tensor_scalar_add(
                                out=tnxt[:, mm, qn * QW:(qn + 1) * QW],
                                in0=ps,
                                scalar1=rhb_s[:, li * 4 + mm:li * 4 + mm + 1])
                    tcur, tnxt = tnxt, tcur
                acur, wb = norm_relu(tcur, "t_b")
                wcol = pt_pool.tile([128, 13], F32, tag="wcol")
                nposs = [128] * 12 + [32]
                for pi in range(13):
                    ps = psum.tile([128, 128], BF16, tag="pst")
                    nc.tensor.transpose(
                        ps[:nposs[pi], :],
                        wb[:, pi * 128:pi * 128 + nposs[pi]], ident)
                    nc.vector.tensor_copy(out=wcol[:nposs[pi], pi:pi + 1],
                                          in_=ps[:nposs[pi], 0:1])
                for pi in range(13):
                    npos = nposs[pi]
                    gt = pt_pool.tile([128, D1], BF16, tag="gtile")
                    for nn2 in range(2):
                        ps = psum.tile([128, 512], F32, tag="ps")
                        for kk in range(4):
                            nc.tensor.matmul(
                                ps[:npos, :],
                                acur[:, kk, pi * 128:pi * 128 + npos],
                                rout_s[:, kk, nn2 * 512:(nn2 + 1) * 512],
                                start=(kk == 0), stop=False)
                        nc.tensor.matmul(
                            ps[:npos, :], ones[0:1, 0:npos],
                            routb_s[:, nn2 * 512:(nn2 + 1) * 512],
                            start=False, stop=True)
                        nc.scalar.activation(
                            out=gt[:npos, nn2 * 512:(nn2 + 1) * 512],
                            in_=ps[:npos, :], func=AF.Copy)
                    nc.sync.dma_start(
                        out=g_dram[pc * PCH + pi * 128:
                                   pc * PCH + pi * 128 + npos, :],
                        in_=gt[:npos, :])
                pt_cm.__exit__(None, None, None)

            rpe_cm.__exit__(None, None, None)

            # ============ norms + xnT ============
            pn = tc.tile_pool(name="pnorm", bufs=1)
            pnp = pn.__enter__()
            xT_s = pnp.tile([128, 4, T], BF16, tag="xT")
            for cc in range(4):
                nc.sync.dma_start(out=xT_s[:, cc, :], in_=dp["xT"][cc])
            Sb = pnp.tile([128, T], BF16, tag="Sb")
            sq = pnp.tile([128, 4, T], BF16, tag="sq")
            for cc in range(4):
                nc.vector.tensor_mul(sq[:, cc, :], xT_s[:, cc, :],
                                     xT_s[:, cc, :])
            for tt in range(NT):
                ps = psum.tile([128, TN], F32, tag="ps")
                for cc in range(4):
                    nc.tensor.matmul(ps, ones,
                                     sq[:, cc, tt * TN:(tt + 1) * TN],
                                     start=(cc == 0), stop=(cc == 3))
                rt = pnp.tile([128, TN], F32, tag="rt")
                nc.scalar.activation(out=rt, in_=ps, func=AF.Sqrt,
                                     scale=float(1.0 / C0))
                st = pnp.tile([128, TN], F32, tag="st")
                nc.vector.tensor_scalar_add(out=st, in0=rt, scalar1=EPS)
                nc.vector.reciprocal(out=Sb[:, tt * TN:(tt + 1) * TN], in_=st)
            for cc in range(4):
                nc.vector.tensor_mul(xnT[:, cc, :], xT_s[:, cc, :], Sb)
            pn.__exit__(None, None, None)

            # ============ Af -> DRAM (c-eighths of 128) ============
            with tc.tile_pool(name="arena", bufs=3) as arena, \
                 tc.tile_pool(name="uv", bufs=2) as uvp, \
                 tc.tile_pool(name="small", bufs=2) as small:
                for ce in range(8):
                    Aw = arena.tile([L2, P, 128], BF16, tag="stage")
                    for pp in range(P):
                        gt = small.tile([P, 128], BF16, tag="g_a1")
                        nc.sync.dma_start(
                            out=gt, in_=g_dram[pp * P:(pp + 1) * P,
                                               ce * 128:(ce + 1) * 128])
                        ps = psum.tile([L2, 128], F32, tag="ps")
                        nc.tensor.matmul(ps, C["GW2"], gt,
                                         start=True, stop=True)
                        nc.scalar.activation(out=Aw[:, pp, :], in_=ps,
                                             func=AF.Copy)
                    T2af = arena.tile([P, 128, L2], BF16, tag="stage")
                    AwT = Aw.rearrange("p q c -> p c q")
                    for ci in range(128):
                        ps = psum.tile([P, L2], BF16, tag="pst")
                        nc.tensor.transpose(ps, AwT[:, ci, :],
                                            ident[:L2, :L2])
                        if ci % 2 == 0:
                            nc.vector.tensor_copy(out=T2af[:, ci, :], in_=ps)
                        else:
                            nc.scalar.activation(out=T2af[:, ci, :], in_=ps,
                                                 func=AF.Copy)
                    T2v = T2af.rearrange("p c (r l) -> p c r l", r=2)
                    for c8 in range(16):
                        for (m1, m2, dst) in ((C["GHc"], C["nGHs"], afr3),
                                              (C["GHs"], C["GHc"], afi3)):
                            ps = psum.tile([P, 8, LW], F32, tag="ps")
                            nc.tensor.matmul(
                                ps, m1, T2v[:, c8 * 8:(c8 + 1) * 8, 0, :],
                                start=True, stop=False)
                            nc.tensor.matmul(
                                ps, m2, T2v[:, c8 * 8:(c8 + 1) * 8, 1, :],
                                start=False, stop=True)
                            ev = small.tile([P, 8, LW], BF16, tag="af_ev")
                            nc.vector.tensor_copy(out=ev, in_=ps)
                            c0g = ce * 128 + c8 * 8
                            nc.sync.dma_start(out=dst[:, c0g:c0g + 8, :],
                                              in_=ev)


                # ============ v-chain per c-eighth ============
                for ce in range(8):
                    u8 = uvp.tile([128, T], BF16, tag="u8")
                    v8 = uvp.tile([128, T], BF16, tag="v8")
                    for (wsrc, bsrc, odst) in ((wu_s, bu_s, u8),
                                               (wv_s, bv_s, v8)):
                        for tt in range(NT):
                            ps = psum.tile([128, TN], F32, tag="ps")
                            for kk in range(4):
                                nc.tensor.matmul(
                                    ps, wsrc[:, kk, ce * 128:(ce + 1) * 128],
                                    xnT[:, kk, tt * TN:(tt + 1) * TN],
                                    start=(kk == 0), stop=(kk == 3))
                            nc.scalar.activation(
                                out=odst[:, tt * TN:(tt + 1) * TN],
                                in_=ps, func=AF.Silu, bias=bsrc[:, ce:ce + 1])
                    # L1 + W-DFT -> Vw (114, n, 128)
                    Vw = arena.tile([L2, N, 128], BF16, tag="stage")
                    for nn in range(N):
                        T1 = small.tile([N, 128], BF16, tag="T1")
                        ps = psum.tile([N, 128], BF16, tag="pst")
                        nc.tensor.transpose(
                            ps, v8[:, nn * N:(nn + 1) * N], ident)
                        nc.vector.tensor_copy(out=T1, in_=ps)
                        ps2 = psum.tile([L2, 128], F32, tag="ps")
                        nc.tensor.matmul(ps2, C["EW2"], T1,
                                         start=True, stop=True)
                        if nn % 2 == 0:
                            nc.vector.tensor_copy(out=Vw[:, nn, :], in_=ps2)
                        else:
                            nc.scalar.activation(out=Vw[:, nn, :], in_=ps2,
                                                 func=AF.Copy)
                    # T2 -> T2big (56n, c, 114)
                    T2big = arena.tile([N, 128, L2], BF16, tag="stage")
                    VwT = Vw.rearrange("p n c -> p c n")
                    for ci in range(128):
                        ps = psum.tile([N, L2], BF16, tag="pst")
                        nc.tensor.transpose(ps, VwT[:, ci, :],
                                            ident[:L2, :L2])
                        if ci % 2 == 0:
                            nc.vector.tensor_copy(out=T2big[:, ci, :], in_=ps)
                        else:
                            nc.scalar.activation(out=T2big[:, ci, :], in_=ps,
                                                 func=AF.Copy)
                    # fwd-H + pointwise -> Pr, Pi
                    Pr = arena.tile([P, 128, LW], BF16, tag="stage")
                    Pi = arena.tile([P, 128, LW], BF16, tag="stage")
                    T2v = T2big.rearrange("p c (r l) -> p c r l", r=2)
                    for c8 in range(16):
                        psr = psum.tile([P, 8, LW], F32, tag="ps")
                        nc.tensor.matmul(psr, C["EHc"],
                                         T2v[:, c8 * 8:(c8 + 1) * 8, 0, :],
                                         start=True, stop=False)
                        nc.tensor.matmul(psr, C["nEHs"],
                                         T2v[:, c8 * 8:(c8 + 1) * 8, 1, :],
                                         start=False, stop=True)
                        psi = psum.tile([P, 8, LW], F32, tag="ps")
                        nc.tensor.matmul(psi, C["EHs"],
                                         T2v[:, c8 * 8:(c8 + 1) * 8, 0, :],
                                         start=True, stop=False)
                        nc.tensor.matmul(psi, C["EHc"],
                                         T2v[:, c8 * 8:(c8 + 1) * 8, 1, :],
                                         start=False, stop=True)
                        cg = ce * 128 + c8 * 8
                        art = small.tile([P, 8, LW], BF16, tag="art")
                        ait = small.tile([P, 8, LW], BF16, tag="ait")
                        nc.sync.dma_start(out=art, in_=afr3[:, cg:cg + 8, :])
                        nc.sync.dma_start(out=ait, in_=afi3[:, cg:cg + 8, :])
                        ta = small.tile([P, 8, LW], BF16, tag="ta")
                        tb = small.tile([P, 8, LW], BF16, tag="tb")
                        prd = Pr[:, c8 * 8:(c8 + 1) * 8, :]
                        pid = Pi[:, c8 * 8:(c8 + 1) * 8, :]
                        nc.vector.tensor_mul(ta, psr, art)
                        nc.vector.tensor_mul(tb, psi, ait)
                        nc.vector.tensor_sub(prd, ta, tb)
                        nc.vector.tensor_mul(ta, psr, ait)
                        nc.vector.tensor_mul(tb, psi, art)
                        nc.vector.tensor_add(pid, ta, tb)
                    # inv-H -> Z
                    Z = arena.tile([P, 128, LW], BF16, tag="stage")
                    for ch in range(16):
                        ps = psum.tile([P, 8, LW], F32, tag="ps")
                        nc.tensor.matmul(ps, C["IH_A"],
                                         Pr[:, ch * 8:(ch + 1) * 8, :],
                                         start=True, stop=False)
                        nc.tensor.matmul(ps, C["IH_B"],
                                         Pi[:, ch * 8:(ch + 1) * 8, :],
                                         start=False, stop=True)
                        if ch % 2 == 0:
                            nc.vector.tensor_copy(
                                out=Z[:, ch * 8:(ch + 1) * 8, :], in_=ps)
                        else:
                            nc.scalar.activation(
                                out=Z[:, ch * 8:(ch + 1) * 8, :], in_=ps,
                                func=AF.Copy)
                    # T3 -> T3big (57l, c, 112)
                    T3big = arena.tile([LW, 128, P], BF16, tag="stage")
                    for ci in range(128):
                        ps = psum.tile([LW, P], BF16, tag="pst")
                        nc.tensor.transpose(ps, Z[:, ci, :], ident[:P, :P])
                        if ci % 2 == 0:
                            nc.vector.tensor_copy(out=T3big[:, ci, :], in_=ps)
                        else:
                            nc.scalar.activation(out=T3big[:, ci, :], in_=ps,
                                                 func=AF.Copy)
                    # inv-W -> Yb (56m, c, 56n)
                    Yb = arena.tile([N, 128, N], BF16, tag="stage")
                    T3r = T3big.rearrange("p c (r n) -> p c r n", r=2)
                    for c8 in range(16):
                        ps = psum.tile([N, 8, N], F32, tag="ps")
                        nc.tensor.matmul(ps, C["IWc"],
                                         T3r[:, c8 * 8:(c8 + 1) * 8, 0, :],
                                         start=True, stop=False)
                        nc.tensor.matmul(ps, C["nIWs"],
                                         T3r[:, c8 * 8:(c8 + 1) * 8, 1, :],
                                         start=False, stop=True)
                        ydst = Yb[:, c8 * 8:(c8 + 1) * 8, :]
                        if c8 % 2 == 0:
                            nc.vector.tensor_copy(out=ydst, in_=ps)
                        else:
                            nc.scalar.activation(out=ydst, in_=ps,
                                                 func=AF.Copy)
                    # T4 + gating -> gat_dram
                    YbT = Yb.rearrange("p c n -> p n c")
                    gq = uvp.tile([128, T], BF16, tag="gq")
                    for nn in range(N):
                        ps = psum.tile([128, N], BF16, tag="pst")
                        nc.tensor.transpose(ps, YbT[:, nn, :], ident[:N, :N])
                        yt = small.tile([128, N], BF16, tag="yt")
                        if nn % 2 == 0:
                            nc.vector.tensor_copy(out=yt, in_=ps)
                        else:
                            nc.scalar.activation(out=yt, in_=ps, func=AF.Copy)
                        nc.vector.tensor_mul(gq[:, nn * N:(nn + 1) * N],
                                             u8[:, nn * N:(nn + 1) * N], yt)
                    nc.sync.dma_start(out=gat_dram[ce], in_=gq)

                # ============ out-proj + bias + residual ============
                for co in range(4):
                    for tt in range(NT):
                        ps = psum.tile([128, TN], F32, tag="ps")
                        for kk in range(8):
                            gk = small.tile([128, TN], BF16, tag="gk")
                            nc.sync.dma_start(
                                out=gk,
                                in_=gat_dram[kk, :, tt * TN:(tt + 1) * TN])
                            nc.tensor.matmul(
                                ps, wo_s[:, kk, co * 128:(co + 1) * 128],
                                gk, start=(kk == 0), stop=(kk == 7))
                        xres = small.tile([128, TN], BF16, tag="xres")
                        nc.sync.dma_start(
                            out=xres,
                            in_=dp["xT"][co, :, tt * TN:(tt + 1) * TN])
                        ot = small.tile([128, TN], F32, tag="ot")
                        nc.vector.scalar_tensor_tensor(
                            out=ot, in0=ps, scalar=bo_s[:, co:co + 1],
                            in1=xres, op0=mybir.AluOpType.add,
                            op1=mybir.AluOpType.add)
                        nc.sync.dma_start(
                            out=outT[co, :, tt * TN:(tt + 1) * TN], in_=ot)

    _hoist_waits(nc)
    return nc


def _hoist_waits(nc):
    for fn in nc.m.functions:
        for bb in fn.blocks:
            new = []
            for inst in bb.instructions:
                si = inst.sync_info
                if (inst.opcode != "EventSemaphore" and si is not None
                        and si.on_wait and len(si.on_wait) > 1):
                    waits = list(si.on_wait)
                    for i, w in enumerate(waits[:-1]):
                        new.append(mybir.InstEventSemaphore(
                            name=f"{inst.name}-hoist{i}",
                            opcode="EventSemaphore",
                            engine=inst.engine,
                            sync_info=mybir.SyncInfo(on_wait=[w],
                                                     on_update=[])))
                    inst.sync_info = mybir.SyncInfo(
                        on_wait=[waits[-1]], on_update=list(si.on_update))
                new.append(inst)
            bb.instructions[:] = new


_NC_CACHE = None


def _get_nc():
    global _NC_CACHE
    if _NC_CACHE is None:
        _NC_CACHE = build_nc()
    return _NC_CACHE


def _cdt(a):
    return np.ascontiguousarray(a).astype(bf)


def kernel(x, W_u, b_u, W_v, b_v, W_o, b_o,
           rpe_in_w, rpe_in_b, rpe_h_w, rpe_h_b, rpe_out_w, rpe_out_b,
           H, W):
    x = np.asarray(x, np.float32)

    def chanfirst(w, nchunk, width):
        # (nchunk*128, width) -> (128, nchunk, width)
        return _cdt(np.ascontiguousarray(
            w.reshape(nchunk, 128, width).transpose(1, 0, 2)))

    common = {
        "wuT": chanfirst(np.ascontiguousarray(W_u.T), 4, D1),
        "wvT": chanfirst(np.ascontiguousarray(W_v.T), 4, D1),
        "woT": chanfirst(np.ascontiguousarray(W_o.T), 8, C0),
        "bu": np.ascontiguousarray(
            np.asarray(b_u, np.float32).reshape(8, 128).T),
        "bv": np.ascontiguousarray(
            np.asarray(b_v, np.float32).reshape(8, 128).T),
        "bo": np.ascontiguousarray(
            np.asarray(b_o, np.float32).reshape(4, 128).T),
        "coordsT": _cdt(_CONSTS["coordsT"]),
        "rpe_inT": _cdt(np.concatenate([rpe_in_w.T, rpe_in_b[None, :]],
                                       axis=0)),
        "rpe_outT": chanfirst(np.ascontiguousarray(rpe_out_w.T), 4, D1),
        "rpe_outB": _cdt(rpe_out_b[None, :]),
        "ident": _cdt(np.eye(128, dtype=np.float32)),
        "ones": _cdt(np.ones((128, 128), np.float32)),
    }
    for i in range(3):
        common[f"rpe_hT{i}"] = chanfirst(
            np.ascontiguousarray(rpe_h_w[i].T), 4, C0)
    common["rpe_hB"] = np.ascontiguousarray(
        np.asarray(rpe_h_b, np.float32).reshape(3, 4, 128)
        .transpose(2, 0, 1).reshape(128, 12))
    for mname in _MATNAMES:
        common[mname] = _cdt(_CONSTS[mname])

    in_maps = []
    for i in range(NCORES):
        m = dict(common)
        m["xT"] = _cdt(x[i].reshape(T, C0).T.reshape(4, 128, T))
        in_maps.append(m)

    nc = _get_nc()
    res = run_bass_kernel_spmd(nc, in_maps, list(range(NCORES)))
    out = np.empty((NCORES, N, N, C0), np.float32)
    for i in range(NCORES):
        o = res.results[i]["outT"].reshape(C0, T)
        out[i] = o.T.reshape(N, N, C0)
    return out


# ----------------------------------------------------------------- benching
def make_device_fn(in_maps):
    """Device-resident jitted callable running the kernel on 8 cores."""
    import jax
    import jax.numpy as jnp
    from jax.sharding import Mesh, PartitionSpec
    from jax.experimental.shard_map import shard_map
    from concourse import bass2jax
    from concourse.bass2jax import _bass_exec_p, partition_id_tensor
    import concourse.mybir as _mybir

    bass2jax.install_neuronx_cc_hook()
    nc = _get_nc()
    partition_name = (nc.partition_id_tensor.name
                      if nc.partition_id_tensor else None)
    in_names, out_names, out_avals, zero_shapes = [], [], [], []
    for alloc in nc.m.functions[0].allocations:
        if not isinstance(alloc, _mybir.MemoryLocationSet):
            continue
        name = alloc.memorylocations[0].name
        if alloc.kind == "ExternalInput":
            if name != partition_name:
                in_names.append(name)
        elif alloc.kind == "ExternalOutput":
            import numpy as _np
            dt = _np.dtype(str(alloc.dtype).split('.')[-1])
            out_avals.append(jax.core.ShapedArray(
                tuple(alloc.tensor_shape), dt))
            out_names.append(name)
            zero_shapes.append((tuple(alloc.tensor_shape), dt))
    all_names = in_names + out_names
    if partition_name is not None:
        all_names.append(partition_name)

    def _body(*args):
        operands = list(args)
        if partition_name is not None:
            operands.append(partition_id_tensor())
        return tuple(_bass_exec_p.bind(
            *operands, out_avals=tuple(out_avals), in_names=tuple(all_names),
            out_names=tuple(out_names), lowering_input_output_aliases=(),
            sim_require_finite=True, sim_require_nnan=True, nc=nc))

    devices = jax.devices()[:NCORES]
    mesh = Mesh(np.asarray(devices), ("core",))
    nz = len(zero_shapes)
    sharded = jax.jit(shard_map(
        _body, mesh=mesh,
        in_specs=(PartitionSpec("core"),) * (len(in_names) + nz),
        out_specs=(PartitionSpec("core"),) * len(out_names),
        check_rep=False), keep_unused=True)
    concat_in = [np.concatenate([np.asarray(in_maps[c][nm])
                                 for c in range(NCORES)], axis=0)
                 for nm in in_names]
    for shp, dt in zero_shapes:
        concat_in.append(np.zeros((NCORES * shp[0],) + tuple(shp[1:]), dt))
    dev_in = [jax.device_put(a) for a in concat_in]
    return sharded, dev_in


def prep_in_maps(x, W_u, b_u, W_v, b_v, W_o, b_o,
                 rpe_in_w, rpe_in_b, rpe_h_w, rpe_h_b,
                 rpe_out_w, rpe_out_b, H=None, W=None):
    """Host-side input marshalling shared by kernel() and benching."""
    x = np.asarray(x, np.float32)

    def chanfirst(w, nchunk, width):
        return _cdt(np.ascontiguousarray(
            w.reshape(nchunk, 128, width).transpose(1, 0, 2)))

    common = {
        "wuT": chanfirst(np.ascontiguousarray(W_u.T), 4, D1),
        "wvT": chanfirst(np.ascontiguousarray(W_v.T), 4, D1),
        "woT": chanfirst(np.ascontiguousarray(W_o.T), 8, C0),
        "bu": np.ascontiguousarray(
            np.asarray(b_u, np.float32).reshape(8, 128).T),
        "bv": np.ascontiguousarray(
            np.asarray(b_v, np.float32).reshape(8, 128).T),
        "bo": np.ascontiguousarray(
            np.asarray(b_o, np.float32).reshape(4, 128).T),
        "coordsT": _cdt(_CONSTS["coordsT"]),
        "rpe_inT": _cdt(np.concatenate([rpe_in_w.T, rpe_in_b[None, :]],
                                       axis=0)),
        "rpe_outT": chanfirst(np.ascontiguousarray(rpe_out_w.T), 4, D1),
        "rpe_outB": _cdt(rpe_out_b[None, :]),
        "ident": _cdt(np.eye(128, dtype=np.float32)),
        "ones": _cdt(np.ones((128, 128), np.float32)),
    }
    for i in range(3):
        common[f"rpe_hT{i}"] = chanfirst(
            np.ascontiguousarray(rpe_h_w[i].T), 4, C0)
    common["rpe_hB"] = np.ascontiguousarray(
        np.asarray(rpe_h_b, np.float32).reshape(3, 4, 128)
        .transpose(2, 0, 1).reshape(128, 12))
    for mname in _MATNAMES:
        common[mname] = _cdt(_CONSTS[mname])
    in_maps = []
    for i in range(NCORES):
        m = dict(common)
        m["xT"] = _cdt(x[i].reshape(T, C0).T.reshape(4, 128, T))
        in_maps.append(m)
    return in_maps



# revision 3
# speedup vs baseline: 2.0399x; 2.0399x over previous
"""Hand-written Bass/Tile kernel for nn_Gtu2d on 8 TRN2 NeuronCores.

Data-parallel over batch (core i owns batch elem i). RPE MLP + Toeplitz
coefficient spectrum (Af) replicated per core (zero collectives),
staged through DRAM. All matmuls bf16 -> f32 PSUM. 2D rFFT/irFFT =
dense DFT matmuls; circulant embedding folded into the coefficient DFT
matrices as a phase shift.

All inputs are packed into a single bf16 blob per core (one DRAM
parameter) to minimize per-argument dispatch overhead; f32 biases are
shipped as hi/lo bf16 pairs and reconstructed on-device. Output is
bf16 on-device, upcast to f32 on host.
"""

import numpy as np
import ml_dtypes

import concourse.bass as bass
import concourse.mybir as mybir
import concourse.tile as tile
from concourse.bass_utils import run_bass_kernel_spmd

BF16 = mybir.dt.bfloat16
F32 = mybir.dt.float32
AF = mybir.ActivationFunctionType

N = 56
T = N * N            # 3136
C0 = 512
D1 = 1024
P = 112
LW = 57
L2 = 114
GRID = P * P         # 12544
NCORES = 8
CQ = 256             # channel quarter
NQ = D1 // CQ
EPS = 1e-8
TN = 448
NT = T // TN

bf = ml_dtypes.bfloat16


def _np_consts():
    n = np.arange(N)
    k = np.arange(P)
    l = np.arange(LW)
    EWc = np.cos(-2 * np.pi * np.outer(n, l) / P).astype(np.float32)
    EWs = np.sin(-2 * np.pi * np.outer(n, l) / P).astype(np.float32)
    EW2 = np.concatenate([EWc, EWs], axis=1)
    EHc = np.cos(-2 * np.pi * np.outer(n, k) / P).astype(np.float32)
    EHs = np.sin(-2 * np.pi * np.outer(n, k) / P).astype(np.float32)
    pi_ = np.arange(P)
    GHc = np.cos(-2 * np.pi * np.outer(pi_ - 55, k) / P).astype(np.float32)
    GHs = np.sin(-2 * np.pi * np.outer(pi_ - 55, k) / P).astype(np.float32)
    GHc[111] = 0.0
    GHs[111] = 0.0
    GWc = np.cos(-2 * np.pi * np.outer(pi_ - 55, l) / P).astype(np.float32)
    GWs = np.sin(-2 * np.pi * np.outer(pi_ - 55, l) / P).astype(np.float32)
    GWc[111] = 0.0
    GWs[111] = 0.0
    GW2 = np.concatenate([GWc, GWs], axis=1)
    wl = np.full(LW, 2.0, np.float32)
    wl[0] = 1.0
    wl[-1] = 1.0
    IHc = np.cos(2 * np.pi * np.outer(k, n) / P).astype(np.float32)
    IHs = np.sin(2 * np.pi * np.outer(k, n) / P).astype(np.float32)
    IWc = (wl[:, None] * np.cos(2 * np.pi * np.outer(l, n) / P) / (P * P)).astype(np.float32)
    IWs = (wl[:, None] * np.sin(2 * np.pi * np.outer(l, n) / P) / (P * P)).astype(np.float32)
    IH_A = np.concatenate([IHc, IHs], axis=1)
    IH_B = np.concatenate([-IHs, IHc], axis=1)
    dp = np.arange(P, dtype=np.float32) - 55.0
    pp, qq = np.meshgrid(dp, dp, indexing='ij')
    coords = np.stack([pp.ravel(), qq.ravel(), np.ones(GRID, np.float32)])
    return dict(EW2=EW2, EHc=EHc, EHs=EHs, nEHs=-EHs, GW2=GW2,
                GHc=GHc, GHs=GHs, nGHs=-GHs,
                IH_A=IH_A, IH_B=IH_B, IWc=IWc, nIWs=-IWs, coordsT=coords)


_CONSTS = _np_consts()
_MATNAMES = ("EW2", "EHc", "EHs", "nEHs", "GW2", "GHc", "GHs", "nGHs",
             "IH_A", "IH_B", "IWc", "nIWs")

# ---------------------------------------------------------------- blob layout
# every input tensor lives in one flat bf16 blob; f32 params are stored as
# hi/lo bf16 pairs (x ~= hi + lo exactly to ~2^-17 rel).
_PIECES = [
    ("xT", (4, 128, T)),
    ("wuT", (128, 4, D1)),
    ("wvT", (128, 4, D1)),
    ("woT", (128, 8, C0)),
    ("coordsT", (3, GRID)),
    ("rpe_inT", (3, C0)),
    ("rpe_hT0", (128, 4, C0)),
    ("rpe_hT1", (128, 4, C0)),
    ("rpe_hT2", (128, 4, C0)),
    ("rpe_outT", (128, 4, D1)),
    ("rpe_outB", (1, D1)),
    ("bu_hi", (128, 8)), ("bu_lo", (128, 8)),
    ("bv_hi", (128, 8)), ("bv_lo", (128, 8)),
    ("bo_hi", (128, 4)), ("bo_lo", (128, 4)),
    ("rpe_hB_hi", (128, 12)), ("rpe_hB_lo", (128, 12)),
    ("ident", (128, 128)),
    ("ones", (128, 128)),
] + [(m, _CONSTS[m].shape) for m in _MATNAMES]

_OFFSET = {}
_BLOB_LEN = 0
for _nm, _shp in _PIECES:
    _OFFSET[_nm] = _BLOB_LEN
    _BLOB_LEN += int(np.prod(_shp))
_PSHAPE = dict(_PIECES)


def _hilo(a):
    a = np.asarray(a, np.float32)
    hi = a.astype(bf)
    lo = (a - hi.astype(np.float32)).astype(bf)
    return hi, lo


def build_nc():
    nc = bass.Bass()
    blob = nc.declare_dram_parameter("cb", [1, _BLOB_LEN], BF16,
                                     isOutput=False)

    def bview(name):
        shp = _PSHAPE[name]
        ofs = _OFFSET[name]
        n_el = int(np.prod(shp))
        ap = blob[0, ofs:ofs + n_el]
        if len(shp) == 2:
            return ap.rearrange("(a b) -> a b", a=shp[0])
        assert len(shp) == 3
        return ap.rearrange("(a b c) -> a b c", a=shp[0], b=shp[1])

    outT = nc.declare_dram_parameter("outT", [4, 128, T], BF16, isOutput=True)

    with tile.TileContext(nc, pool_alloc_mode="queue") as tc:
        with nc.allow_low_precision(reason="bf16 kernel, 2e-2 tolerance"), \
             tc.tile_pool(name="consts", bufs=1) as consts, \
             tc.tile_pool(name="persist", bufs=1) as persist, \
             tc.tile_pool(name="psum", bufs=4, space="PSUM") as psum, \
             tc.tile_pool(name="dram", bufs=1, space="DRAM") as dram:

            def cload(name, dt=BF16):
                t = consts.tile(list(_PSHAPE[name]), dt, tag=name)
                nc.sync.dma_start(out=t, in_=bview(name)[:])
                return t

            def f32load(name, shape):
                hi = consts.tile(list(shape), BF16, tag=name + "_hi")
                nc.sync.dma_start(out=hi, in_=bview(name + "_hi")[:])
                lo = consts.tile(list(shape), BF16, tag=name + "_lo")
                nc.sync.dma_start(out=lo, in_=bview(name + "_lo")[:])
                t = consts.tile(list(shape), F32, tag=name)
                nc.vector.tensor_add(t, hi, lo)
                return t

            C = {m: cload(m) for m in _MATNAMES}
            ident = cload("ident")
            ones = cload("ones")
            wu_s = cload("wuT")
            wv_s = cload("wvT")
            wo_s = cload("woT")
            bu_s = f32load("bu", (128, 8))
            bv_s = f32load("bv", (128, 8))
            bo_s = f32load("bo", (128, 4))

            xnT = persist.tile([128, 4, T], BF16, tag="xnT")

            g_dram = dram.tile([GRID, D1], BF16, tag="g_dram")
            afr_dram = dram.tile([P, D1 * LW], BF16, tag="afr_dram")
            afi_dram = dram.tile([P, D1 * LW], BF16, tag="afi_dram")
            gat_dram = dram.tile([8, 128, T], BF16, tag="gat_dram")
            afr3 = afr_dram.rearrange("p (c l) -> p c l", l=LW)
            afi3 = afi_dram.rearrange("p (c l) -> p c l", l=LW)

            # ============ RPE trunk -> g_dram ============
            PCH = 1568
            QN = 4
            QW = PCH // QN
            rpe_cm = tc.tile_pool(name="rpew", bufs=1)
            rpw = rpe_cm.__enter__()

            def rload(name, dt=BF16):
                t = rpw.tile(list(_PSHAPE[name]), dt, tag=name)
                nc.sync.dma_start(out=t, in_=bview(name)[:])
                return t

            rin_s = rload("rpe_inT")
            rh_s = [rload(f"rpe_hT{i}") for i in range(3)]
            rout_s = rload("rpe_outT")
            routb_s = rload("rpe_outB")
            rhb_hi = rpw.tile([128, 12], BF16, tag="rhb_hi")
            nc.sync.dma_start(out=rhb_hi, in_=bview("rpe_hB_hi")[:])
            rhb_lo = rpw.tile([128, 12], BF16, tag="rhb_lo")
            nc.sync.dma_start(out=rhb_lo, in_=bview("rpe_hB_lo")[:])
            rhb_s = rpw.tile([128, 12], F32, tag="rhb")
            nc.vector.tensor_add(rhb_s, rhb_hi, rhb_lo)
            for pc in range(GRID // PCH):
                pt_cm = tc.tile_pool(name="ptrunk", bufs=1)
                pt_pool = pt_cm.__enter__()
                cs = pt_pool.tile([3, PCH], BF16, tag="coords")
                nc.sync.dma_start(
                    out=cs, in_=bview("coordsT")[:, pc * PCH:(pc + 1) * PCH])
                tcur = pt_pool.tile([128, 4, PCH], BF16, tag="t_a")
                for mm in range(4):
                    for qn in range(QN):
                        ps = psum.tile([128, QW], F32, tag="ps")
                        nc.tensor.matmul(
                            ps, rin_s[:, mm * 128:(mm + 1) * 128],
                            cs[:, qn * QW:(qn + 1) * QW],
                            start=True, stop=True)
                        nc.scalar.activation(
                            out=tcur[:, mm, qn * QW:(qn + 1) * QW],
                            in_=ps, func=AF.Copy)

                def norm_relu(tin, tag):
                    s0 = pt_pool.tile([128, PCH], BF16, tag="nsq0")
                    tmp = pt_pool.tile([128, PCH], BF16, tag="nsq1")
                    nc.vector.tensor_mul(s0, tin[:, 0, :], tin[:, 0, :])
                    for cc in range(1, 4):
                        nc.vector.tensor_mul(tmp, tin[:, cc, :], tin[:, cc, :])
                        nc.vector.tensor_add(s0, s0, tmp)
                    wb = pt_pool.tile([128, PCH], BF16, tag="wb")
                    for qn in range(QN):
                        ps = psum.tile([128, QW], F32, tag="ps")
                        nc.tensor.matmul(ps, ones,
                                         s0[:, qn * QW:(qn + 1) * QW],
                                         start=True, stop=True)
                        rt = pt_pool.tile([128, QW], F32, tag="nrt")
                        nc.scalar.activation(out=rt, in_=ps, func=AF.Sqrt,
                                             scale=float(1.0 / C0))
                        nc.vector.reciprocal(
                            out=wb[:, qn * QW:(qn + 1) * QW], in_=rt)
                    acur = pt_pool.tile([128, 4, PCH], BF16, tag=tag)
                    for cc in range(4):
                        rl = pt_pool.tile([128, PCH], BF16, tag="nrl")
                        nc.scalar.activation(out=rl, in_=tin[:, cc, :],
                                             func=AF.Relu)
                        nc.vector.tensor_mul(acur[:, cc, :], rl, wb)
                    return acur, wb

                for li in range(3):
                    acur, _ = norm_relu(tcur, "t_b")
                    tnxt = pt_pool.tile([128, 4, PCH], BF16, tag="t_c")
                    for mm in range(4):
                        for qn in range(QN):
                            ps = psum.tile([128, QW], F32, tag="ps")
                            for kk in range(4):
                                nc.tensor.matmul(
                                    ps,
                                    rh_s[li][:, kk, mm * 128:(mm + 1) * 128],
                                    acur[:, kk, qn * QW:(qn + 1) * QW],
                                    start=(kk == 0), stop=(kk == 3))
                            nc.vector.tensor_scalar_add(
                                out=tnxt[:, mm, qn * QW:(qn + 1) * QW],
                                in0=ps,
                                scalar1=rhb_s[:, li * 4 + mm:li * 4 + mm + 1])
                    tcur, tnxt = tnxt, tcur
                acur, wb = norm_relu(tcur, "t_b")
                wcol = pt_pool.tile([128, 13], F32, tag="wcol")
                nposs = [128] * 12 + [32]
                for pi in range(13):
                    ps = psum.tile([128, 128], BF16, tag="pst")
                    nc.tensor.transpose(
                        ps[:nposs[pi], :],
                        wb[:, pi * 128:pi * 128 + nposs[pi]], ident)
                    nc.vector.tensor_copy(out=wcol[:nposs[pi], pi:pi + 1],
                                          in_=ps[:nposs[pi], 0:1])
                for pi in range(13):
                    npos = nposs[pi]
                    gt = pt_pool.tile([128, D1], BF16, tag="gtile")
                    for nn2 in range(2):
                        ps = psum.tile([128, 512], F32, tag="ps")
                        for kk in range(4):
                            nc.tensor.matmul(
                                ps[:npos, :],
                                acur[:, kk, pi * 128:pi * 128 + npos],
                                rout_s[:, kk, nn2 * 512:(nn2 + 1) * 512],
                                start=(kk == 0), stop=False)
                        nc.tensor.matmul(
                            ps[:npos, :], ones[0:1, 0:npos],
                            routb_s[:, nn2 * 512:(nn2 + 1) * 512],
                            start=False, stop=True)
                        nc.scalar.activation(
                            out=gt[:npos, nn2 * 512:(nn2 + 1) * 512],
                            in_=ps[:npos, :], func=AF.Copy)
                    nc.sync.dma_start(
                        out=g_dram[pc * PCH + pi * 128:
                                   pc * PCH + pi * 128 + npos, :],
                        in_=gt[:npos, :])
                pt_cm.__exit__(None, None, None)

            rpe_cm.__exit__(None, None, None)

            # ============ norms + xnT ============
            pn = tc.tile_pool(name="pnorm", bufs=1)
            pnp = pn.__enter__()
            xT_s = pnp.tile([128, 4, T], BF16, tag="xT")
            for cc in range(4):
                nc.sync.dma_start(out=xT_s[:, cc, :], in_=bview("xT")[cc])
            Sb = pnp.tile([128, T], BF16, tag="Sb")
            sq = pnp.tile([128, 4, T], BF16, tag="sq")
            for cc in range(4):
                nc.vector.tensor_mul(sq[:, cc, :], xT_s[:, cc, :],
                                     xT_s[:, cc, :])
            for tt in range(NT):
                ps = psum.tile([128, TN], F32, tag="ps")
                for cc in range(4):
                    nc.tensor.matmul(ps, ones,
                                     sq[:, cc, tt * TN:(tt + 1) * TN],
                                     start=(cc == 0), stop=(cc == 3))
                rt = pnp.tile([128, TN], F32, tag="rt")
                nc.scalar.activation(out=rt, in_=ps, func=AF.Sqrt,
                                     scale=float(1.0 / C0))
                st = pnp.tile([128, TN], F32, tag="st")
                nc.vector.tensor_scalar_add(out=st, in0=rt, scalar1=EPS)
                nc.vector.reciprocal(out=Sb[:, tt * TN:(tt + 1) * TN], in_=st)
            for cc in range(4):
                nc.vector.tensor_mul(xnT[:, cc, :], xT_s[:, cc, :], Sb)
            pn.__exit__(None, None, None)

            # ============ Af -> DRAM (c-eighths of 128) ============
            with tc.tile_pool(name="arena", bufs=3) as arena, \
                 tc.tile_pool(name="uv", bufs=2) as uvp, \
                 tc.tile_pool(name="small", bufs=2) as small:
                for ce in range(8):
                    Aw = arena.tile([L2, P, 128], BF16, tag="stage")
                    for pp in range(P):
                        gt = small.tile([P, 128], BF16, tag="g_a1")
                        nc.sync.dma_start(
                            out=gt, in_=g_dram[pp * P:(pp + 1) * P,
                                               ce * 128:(ce + 1) * 128])
                        ps = psum.tile([L2, 128], F32, tag="ps")
                        nc.tensor.matmul(ps, C["GW2"], gt,
                                         start=True, stop=True)
                        nc.scalar.activation(out=Aw[:, pp, :], in_=ps,
                                             func=AF.Copy)
                    T2af = arena.tile([P, 128, L2], BF16, tag="stage")
                    AwT = Aw.rearrange("p q c -> p c q")
                    for ci in range(128):
                        ps = psum.tile([P, L2], BF16, tag="pst")
                        nc.tensor.transpose(ps, AwT[:, ci, :],
                                            ident[:L2, :L2])
                        if ci % 2 == 0:
                            nc.vector.tensor_copy(out=T2af[:, ci, :], in_=ps)
                        else:
                            nc.scalar.activation(out=T2af[:, ci, :], in_=ps,
                                                 func=AF.Copy)
                    T2v = T2af.rearrange("p c (r l) -> p c r l", r=2)
                    for c8 in range(16):
                        for (m1, m2, dst) in ((C["GHc"], C["nGHs"], afr3),
                                              (C["GHs"], C["GHc"], afi3)):
                            ps = psum.tile([P, 8, LW], F32, tag="ps")
                            nc.tensor.matmul(
                                ps, m1, T2v[:, c8 * 8:(c8 + 1) * 8, 0, :],
                                start=True, stop=False)
                            nc.tensor.matmul(
                                ps, m2, T2v[:, c8 * 8:(c8 + 1) * 8, 1, :],
                                start=False, stop=True)
                            ev = small.tile([P, 8, LW], BF16, tag="af_ev")
                            nc.vector.tensor_copy(out=ev, in_=ps)
                            c0g = ce * 128 + c8 * 8
                            nc.sync.dma_start(out=dst[:, c0g:c0g + 8, :],
                                              in_=ev)


                # ============ v-chain per c-eighth ============
                for ce in range(8):
                    u8 = uvp.tile([128, T], BF16, tag="u8")
                    v8 = uvp.tile([128, T], BF16, tag="v8")
                    for (wsrc, bsrc, odst) in ((wu_s, bu_s, u8),
                                               (wv_s, bv_s, v8)):
                        for tt in range(NT):
                            ps = psum.tile([128, TN], F32, tag="ps")
                            for kk in range(4):
                                nc.tensor.matmul(
                                    ps, wsrc[:, kk, ce * 128:(ce + 1) * 128],
                                    xnT[:, kk, tt * TN:(tt + 1) * TN],
                                    start=(kk == 0), stop=(kk == 3))
                            nc.scalar.activation(
                                out=odst[:, tt * TN:(tt + 1) * TN],
                                in_=ps, func=AF.Silu, bias=bsrc[:, ce:ce + 1])
                    # L1 + W-DFT -> Vw (114, n, 128)
                    Vw = arena.tile([L2, N, 128], BF16, tag="stage")
                    for nn in range(N):
                        T1 = small.tile([N, 128], BF16, tag="T1")
                        ps = psum.tile([N, 128], BF16, tag="pst")
                        nc.tensor.transpose(
                            ps, v8[:, nn * N:(nn + 1) * N], ident)
                        nc.vector.tensor_copy(out=T1, in_=ps)
                        ps2 = psum.tile([L2, 128], F32, tag="ps")
                        nc.tensor.matmul(ps2, C["EW2"], T1,
                                         start=True, stop=True)
                        if nn % 2 == 0:
                            nc.vector.tensor_copy(out=Vw[:, nn, :], in_=ps2)
                        else:
                            nc.scalar.activation(out=Vw[:, nn, :], in_=ps2,
                                                 func=AF.Copy)
                    # T2 -> T2big (56n, c, 114)
                    T2big = arena.tile([N, 128, L2], BF16, tag="stage")
                    VwT = Vw.rearrange("p n c -> p c n")
                    for ci in range(128):
                        ps = psum.tile([N, L2], BF16, tag="pst")
                        nc.tensor.transpose(ps, VwT[:, ci, :],
                                            ident[:L2, :L2])
                        if ci % 2 == 0:
                            nc.vector.tensor_copy(out=T2big[:, ci, :], in_=ps)
                        else:
                            nc.scalar.activation(out=T2big[:, ci, :], in_=ps,
                                                 func=AF.Copy)
                    # fwd-H + pointwise -> Pr, Pi
                    Pr = arena.tile([P, 128, LW], BF16, tag="stage")
                    Pi = arena.tile([P, 128, LW], BF16, tag="stage")
                    T2v = T2big.rearrange("p c (r l) -> p c r l", r=2)
                    for c8 in range(16):
                        psr = psum.tile([P, 8, LW], F32, tag="ps")
                        nc.tensor.matmul(psr, C["EHc"],
                                         T2v[:, c8 * 8:(c8 + 1) * 8, 0, :],
                                         start=True, stop=False)
                        nc.tensor.matmul(psr, C["nEHs"],
                                         T2v[:, c8 * 8:(c8 + 1) * 8, 1, :],
                                         start=False, stop=True)
                        psi = psum.tile([P, 8, LW], F32, tag="ps")
                        nc.tensor.matmul(psi, C["EHs"],
                                         T2v[:, c8 * 8:(c8 + 1) * 8, 0, :],
                                         start=True, stop=False)
                        nc.tensor.matmul(psi, C["EHc"],
                                         T2v[:, c8 * 8:(c8 + 1) * 8, 1, :],
                                         start=False, stop=True)
                        cg = ce * 128 + c8 * 8
                        art = small.tile([P, 8, LW], BF16, tag="art")
                        ait = small.tile([P, 8, LW], BF16, tag="ait")
                        nc.sync.dma_start(out=art, in_=afr3[:, cg:cg + 8, :])
                        nc.sync.dma_start(out=ait, in_=afi3[:, cg:cg + 8, :])
                        ta = small.tile([P, 8, LW], BF16, tag="ta")
                        tb = small.tile([P, 8, LW], BF16, tag="tb")
                        prd = Pr[:, c8 * 8:(c8 + 1) * 8, :]
                        pid = Pi[:, c8 * 8:(c8 + 1) * 8, :]
                        nc.vector.tensor_mul(ta, psr, art)
                        nc.vector.tensor_mul(tb, psi, ait)
                        nc.vector.tensor_sub(prd, ta, tb)
                        nc.vector.tensor_mul(ta, psr, ait)
                        nc.vector.tensor_mul(tb, psi, art)
                        nc.vector.tensor_add(pid, ta, tb)
                    # inv-H -> Z
                    Z = arena.tile([P, 128, LW], BF16, tag="stage")
                    for ch in range(16):
                        ps = psum.tile([P, 8, LW], F32, tag="ps")
                        nc.tensor.matmul(ps, C["IH_A"],
                                         Pr[:, ch * 8:(ch + 1) * 8, :],
                                         start=True, stop=False)
                        nc.tensor.matmul(ps, C["IH_B"],
                                         Pi[:, ch * 8:(ch + 1) * 8, :],
                                         start=False, stop=True)
                        if ch % 2 == 0:
                            nc.vector.tensor_copy(
                                out=Z[:, ch * 8:(ch + 1) * 8, :], in_=ps)
                        else:
                            nc.scalar.activation(
                                out=Z[:, ch * 8:(ch + 1) * 8, :], in_=ps,
                                func=AF.Copy)
                    # T3 -> T3big (57l, c, 112)
                    T3big = arena.tile([LW, 128, P], BF16, tag="stage")
                    for ci in range(128):
                        ps = psum.tile([LW, P], BF16, tag="pst")
                        nc.tensor.transpose(ps, Z[:, ci, :], ident[:P, :P])
                        if ci % 2 == 0:
                            nc.vector.tensor_copy(out=T3big[:, ci, :], in_=ps)
                        else:
                            nc.scalar.activation(out=T3big[:, ci, :], in_=ps,
                                                 func=AF.Copy)
                    # inv-W -> Yb (56m, c, 56n)
                    Yb = arena.tile([N, 128, N], BF16, tag="stage")
                    T3r = T3big.rearrange("p c (r n) -> p c r n", r=2)
                    for c8 in range(16):
                        ps = psum.tile([N, 8, N], F32, tag="ps")
                        nc.tensor.matmul(ps, C["IWc"],
                                         T3r[:, c8 * 8:(c8 + 1) * 8, 0, :],
                                         start=True, stop=False)
                        nc.tensor.matmul(ps, C["nIWs"],
                                         T3r[:, c8 * 8:(c8 + 1) * 8, 1, :],
                                         start=False, stop=True)
                        ydst = Yb[:, c8 * 8:(c8 + 1) * 8, :]
                        if c8 % 2 == 0:
                            nc.vector.tensor_copy(out=ydst, in_=ps)
                        else:
                            nc.scalar.activation(out=ydst, in_=ps,
                                                 func=AF.Copy)
                    # T4 + gating -> gat_dram
                    YbT = Yb.rearrange("p c n -> p n c")
                    gq = uvp.tile([128, T], BF16, tag="gq")
                    for nn in range(N):
                        ps = psum.tile([128, N], BF16, tag="pst")
                        nc.tensor.transpose(ps, YbT[:, nn, :], ident[:N, :N])
                        yt = small.tile([128, N], BF16, tag="yt")
                        if nn % 2 == 0:
                            nc.vector.tensor_copy(out=yt, in_=ps)
                        else:
                            nc.scalar.activation(out=yt, in_=ps, func=AF.Copy)
                        nc.vector.tensor_mul(gq[:, nn * N:(nn + 1) * N],
                                             u8[:, nn * N:(nn + 1) * N], yt)
                    nc.sync.dma_start(out=gat_dram[ce], in_=gq)

                # ============ out-proj + bias + residual ============
                for co in range(4):
                    for tt in range(NT):
                        ps = psum.tile([128, TN], F32, tag="ps")
                        for kk in range(8):
                            gk = small.tile([128, TN], BF16, tag="gk")
                            nc.sync.dma_start(
                                out=gk,
                                in_=gat_dram[kk, :, tt * TN:(tt + 1) * TN])
                            nc.tensor.matmul(
                                ps, wo_s[:, kk, co * 128:(co + 1) * 128],
                                gk, start=(kk == 0), stop=(kk == 7))
                        xres = small.tile([128, TN], BF16, tag="xres")
                        nc.sync.dma_start(
                            out=xres,
                            in_=bview("xT")[co, :, tt * TN:(tt + 1) * TN])
                        ot = small.tile([128, TN], BF16, tag="ot")
                        nc.vector.scalar_tensor_tensor(
                            out=ot, in0=ps, scalar=bo_s[:, co:co + 1],
                            in1=xres, op0=mybir.AluOpType.add,
                            op1=mybir.AluOpType.add)
                        nc.sync.dma_start(
                            out=outT[co, :, tt * TN:(tt + 1) * TN], in_=ot)

    _hoist_waits(nc)
    return nc


def _hoist_waits(nc):
    for fn in nc.m.functions:
        for bb in fn.blocks:
            new = []
            for inst in bb.instructions:
                si = inst.sync_info
                if (inst.opcode != "EventSemaphore" and si is not None
                        and si.on_wait and len(si.on_wait) > 1):
                    waits = list(si.on_wait)
                    for i, w in enumerate(waits[:-1]):
                        new.append(mybir.InstEventSemaphore(
                            name=f"{inst.name}-hoist{i}",
                            opcode="EventSemaphore",
                            engine=inst.engine,
                            sync_info=mybir.SyncInfo(on_wait=[w],
                                                     on_update=[])))
                    inst.sync_info = mybir.SyncInfo(
                        on_wait=[waits[-1]], on_update=list(si.on_update))
                new.append(inst)
            bb.instructions[:] = new


_NC_CACHE = None


def _get_nc():
    global _NC_CACHE
    if _NC_CACHE is None:
        _NC_CACHE = build_nc()
    return _NC_CACHE


def _cdt(a):
    return np.ascontiguousarray(a).astype(bf)


def _pack_common(W_u, b_u, W_v, b_v, W_o, b_o,
                 rpe_in_w, rpe_in_b, rpe_h_w, rpe_h_b,
                 rpe_out_w, rpe_out_b):
    """Fill the blob (except the per-core xT region) and return it."""
    def chanfirst(w, nchunk, width):
        return _cdt(np.ascontiguousarray(
            w.reshape(nchunk, 128, width).transpose(1, 0, 2)))

    pieces = {
        "wuT": chanfirst(np.ascontiguousarray(W_u.T), 4, D1),
        "wvT": chanfirst(np.ascontiguousarray(W_v.T), 4, D1),
        "woT": chanfirst(np.ascontiguousarray(W_o.T), 8, C0),
        "coordsT": _cdt(_CONSTS["coordsT"]),
        "rpe_inT": _cdt(np.concatenate([rpe_in_w.T, rpe_in_b[None, :]],
                                       axis=0)),
        "rpe_outT": chanfirst(np.ascontiguousarray(rpe_out_w.T), 4, D1),
        "rpe_outB": _cdt(rpe_out_b[None, :]),
        "ident": _cdt(np.eye(128, dtype=np.float32)),
        "ones": _cdt(np.ones((128, 128), np.float32)),
    }
    for i in range(3):
        pieces[f"rpe_hT{i}"] = chanfirst(
            np.ascontiguousarray(rpe_h_w[i].T), 4, C0)
    for mname in _MATNAMES:
        pieces[mname] = _cdt(_CONSTS[mname])

    f32s = {
        "bu": np.ascontiguousarray(
            np.asarray(b_u, np.float32).reshape(8, 128).T),
        "bv": np.ascontiguousarray(
            np.asarray(b_v, np.float32).reshape(8, 128).T),
        "bo": np.ascontiguousarray(
            np.asarray(b_o, np.float32).reshape(4, 128).T),
        "rpe_hB": np.ascontiguousarray(
            np.asarray(rpe_h_b, np.float32).reshape(3, 4, 128)
            .transpose(2, 0, 1).reshape(128, 12)),
    }
    for nm, arr in f32s.items():
        hi, lo = _hilo(arr)
        pieces[nm + "_hi"] = hi
        pieces[nm + "_lo"] = lo

    blob = np.zeros(_BLOB_LEN, dtype=bf)
    for nm, arr in pieces.items():
        ofs = _OFFSET[nm]
        n_el = int(np.prod(_PSHAPE[nm]))
        assert arr.size == n_el, (nm, arr.shape, _PSHAPE[nm])
        blob[ofs:ofs + n_el] = arr.ravel()
    return blob


def prep_in_maps(x, W_u, b_u, W_v, b_v, W_o, b_o,
                 rpe_in_w, rpe_in_b, rpe_h_w, rpe_h_b,
                 rpe_out_w, rpe_out_b, H=None, W=None):
    """Host-side input marshalling shared by kernel() and benching."""
    x = np.asarray(x, np.float32)
    blob0 = _pack_common(W_u, b_u, W_v, b_v, W_o, b_o,
                         rpe_in_w, rpe_in_b, rpe_h_w, rpe_h_b,
                         rpe_out_w, rpe_out_b)
    xofs = _OFFSET["xT"]
    xlen = int(np.prod(_PSHAPE["xT"]))
    in_maps = []
    for i in range(NCORES):
        b = blob0.copy()
        b[xofs:xofs + xlen] = _cdt(
            x[i].reshape(T, C0).T.reshape(4, 128, T)).ravel()
        in_maps.append({"cb": b.reshape(1, _BLOB_LEN)})
    return in_maps


def kernel(x, W_u, b_u, W_v, b_v, W_o, b_o,
           rpe_in_w, rpe_in_b, rpe_h_w, rpe_h_b, rpe_out_w, rpe_out_b,
           H, W):
    in_maps = prep_in_maps(x, W_u, b_u, W_v, b_v, W_o, b_o,
                           rpe_in_w, rpe_in_b, rpe_h_w, rpe_h_b,
                           rpe_out_w, rpe_out_b)
    nc = _get_nc()
    res = run_bass_kernel_spmd(nc, in_maps, list(range(NCORES)))
    out = np.empty((NCORES, N, N, C0), np.float32)
    for i in range(NCORES):
        o = res.results[i]["outT"].astype(np.float32).reshape(C0, T)
        out[i] = o.T.reshape(N, N, C0)
    return out


# ----------------------------------------------------------------- benching
def make_device_fn(in_maps):
    """Device-resident jitted callable running the kernel on 8 cores."""
    import jax
    import jax.numpy as jnp
    from jax.sharding import Mesh, PartitionSpec
    from jax.experimental.shard_map import shard_map
    from concourse import bass2jax
    from concourse.bass2jax import _bass_exec_p, partition_id_tensor
    import concourse.mybir as _mybir

    bass2jax.install_neuronx_cc_hook()
    nc = _get_nc()
    partition_name = (nc.partition_id_tensor.name
                      if nc.partition_id_tensor else None)
    in_names, out_names, out_avals, zero_shapes = [], [], [], []
    for alloc in nc.m.functions[0].allocations:
        if not isinstance(alloc, _mybir.MemoryLocationSet):
            continue
        name = alloc.memorylocations[0].name
        if alloc.kind == "ExternalInput":
            if name != partition_name:
                in_names.append(name)
        elif alloc.kind == "ExternalOutput":
            import numpy as _np
            _dtname = str(alloc.dtype).split('.')[-1]
            try:
                dt = _np.dtype(_dtname)
            except TypeError:
                dt = _np.dtype(getattr(ml_dtypes, _dtname))
            out_avals.append(jax.core.ShapedArray(
                tuple(alloc.tensor_shape), dt))
            out_names.append(name)
            zero_shapes.append((tuple(alloc.tensor_shape), dt))
    all_names = in_names + out_names
    if partition_name is not None:
        all_names.append(partition_name)

    def _body(*args):
        operands = list(args)
        if partition_name is not None:
            operands.append(partition_id_tensor())
        return tuple(_bass_exec_p.bind(
            *operands, out_avals=tuple(out_avals), in_names=tuple(all_names),
            out_names=tuple(out_names), lowering_input_output_aliases=(),
            sim_require_finite=True, sim_require_nnan=True, nc=nc))

    devices = jax.devices()[:NCORES]
    mesh = Mesh(np.asarray(devices), ("core",))
    nz = len(zero_shapes)
    sharded = jax.jit(shard_map(
        _body, mesh=mesh,
        in_specs=(PartitionSpec("core"),) * (len(in_names) + nz),
        out_specs=(PartitionSpec("core"),) * len(out_names),
        check_rep=False), keep_unused=True)
    concat_in = [np.concatenate([np.asarray(in_maps[c][nm])
                                 for c in range(NCORES)], axis=0)
                 for nm in in_names]
    for shp, dt in zero_shapes:
        concat_in.append(np.zeros((NCORES * shp[0],) + tuple(shp[1:]), dt))
    dev_in = [jax.device_put(a) for a in concat_in]
    return sharded, dev_in
